# revision 2
# baseline (speedup 1.0000x reference)
"""Trainium2 Bass kernel for nn_DistanceDecoder (moe_routing).

reference:
    comp_b  = components[object_labels]            # [B, 32, 6144]
    mean_b  = means[object_labels]                 # [B, 6144]
    out     = einsum('bp,bpo->bo', lattent, comp_b) + mean_b

Strategy (8 NeuronCores):
  * Shard OUT_DIM (6144) 8-ways -> each core owns a 768-wide column slice
    and the full batch.  Per-core HBM traffic is then ~2.8 MB (its own
    fp16 slice of the PCA table + 1.5 MB fp16 output) instead of the
    18 MB the batch-parallel split would need.
  * On the host, stable-sort the batch by label (MoE dispatch) and append
    the per-object mean as a 33rd row of each object's [32, 768] component
    block with a matching constant-1.0 row in the latent matrix, so gather
    + vecmat + mean-add is a single block-banded matmul
        out_T[768, 1024] = C2aug^T @ Epack
    over 7 K-tiles of 3 objects (K = 3*33 = 99 rows).  After the sort,
    each K-tile's samples form one contiguous column range, baked into the
    instruction stream as matmul free-dim offsets.
  * Matmul operands are fp16 (~10-bit-mantissa rounding, full-rate PE,
    half the DMA bytes).  The output is also stored fp16 (cast during the
    PSUM->SBUF drain, upcast on the host): adds ~2e-4 rel err and halves
    the store traffic.
  * No matmul output range may cross the 512-column PSUM bank boundary:
    on the FIRST execution after process start (cold PE p-state) a
    bank-crossing matmul's writes to the second bank are corrupted
    (observed: exactly the >=512 part of the one straddling K-tile range,
    off by O(1) per element, later runs clean).  Ranges straddling 512
    are therefore split into two matmuls at 512.
  * Profiling showed HBM READS are the wall (~110 GB/s per DMA queue vs
    ~350-400 GB/s for writes; the 16 DMA engines aggregate ~390 GB/s),
    so loads are spread over THREE queues: the two HWDGE rings (sync,
    scalar) plus the Pool-engine SWDGE queue (gpsimd.dma_start).  The
    host packs comp so every per-chunk load reads one fully CONTIGUOUS
    177 KB block ([NCHUNK*KP, CCOLS] layout).  Epack is split at column
    512 across two queues so the first K-tiles' matmuls (ranges < 512)
    only wait for the first half.  All loads are enqueued before any
    store on the same ring (a store's semaphore wait would head-of-line
    block later loads).
  * Stores are per-chunk (contiguous 256 KB in [SLICE, BATCH] layout) on
    the two HWDGE rings, each emitted right after its chunk's PSUM drain
    so its semaphore wait is already resolved.  The last chunk drains and
    stores per 512-segment to shorten the final tail.
  * PSUM drains are one [128,1024] f32->fp16 copy per chunk (2 PSUM
    banks, pool of 4), alternating DVE / ACT.
  * kernel() validates the result on the host (column-sum invariant over
    the whole output + 32 exact rows at stride-32 sorted positions; both
    far above fp16 noise) and re-runs on mismatch: the first execution
    after process start is occasionally corrupted by the axon/PJRT path
    even with the bank-split fix, and a re-run has always been clean.
  * DD_DTYPE=f32r swaps in fp32r (full fp32 operand bits, f32 output);
    fp32r matmuls then require even range starts/widths, fixed by zero
    pad columns, with samples pushed past column 1024 computed on the
    host.
  * Host applies the inverse permutation / column concat at the end.
"""

import os

import numpy as np

BATCH = 1024
PCA = 32
ROWS = PCA + 1             # 32 components + 1 mean row per object
OUT_DIM = 6144
NOBJ = 20
NCORES = 8
SLICE = OUT_DIM // NCORES  # 768
NCHUNK = SLICE // 128      # 6 chunks of 128 output rows (out_T partitions)
OBJ_PER_KT = 3             # objects per K-tile -> K = 3*33 = 99 <= 128
KTILES = (NOBJ + OBJ_PER_KT - 1) // OBJ_PER_KT  # 7
KP = OBJ_PER_KT * ROWS     # 99 partitions per K-tile
BANK = 512                 # PSUM bank boundary (f32 cols)

DTYPE = os.environ.get("DD_DTYPE", "fp16")  # "fp16" | "f32r"

_NC_CACHE: dict = {}


def _kheight(t: int) -> int:
    return (min(OBJ_PER_KT * (t + 1), NOBJ) - OBJ_PER_KT * t) * ROWS


def _np_dtype():
    return np.float16 if DTYPE == "fp16" else np.float32


def _build_nc(ranges: tuple):
    """Build + compile the single-core Bass program (SPMD across 8 cores).

    ranges: KTILES+1 ints; ranges[t]..ranges[t+1] is the sorted-batch column
    range whose labels fall in objects [3t, 3t+3) — baked into the
    instruction stream as matmul free-dim offsets.
    """
    import concourse.mybir as mybir
    from concourse import bacc
    from concourse.tile import TileContext

    dt_in = mybir.dt.float16 if DTYPE == "fp16" else mybir.dt.float32r
    dt_out = mybir.dt.float16 if DTYPE == "fp16" else mybir.dt.float32
    f32 = mybir.dt.float32

    nc = bacc.Bacc("TRN2", target_bir_lowering=False, debug=False)

    # comp packed per chunk: rows [j*KP, (j+1)*KP) = chunk j, contiguous
    CCOLS = KTILES * 128
    comp_d = nc.dram_tensor(
        "comp", [NCHUNK * KP, CCOLS], dt_in, kind="ExternalInput"
    )
    epack_d = nc.dram_tensor("epack", [KP, BATCH], dt_in, kind="ExternalInput")
    out_d = nc.dram_tensor("out", [SLICE, BATCH], dt_out, kind="ExternalOutput")

    with TileContext(nc) as tc:
        with (
            tc.tile_pool(name="sb", bufs=1) as cpool,
            tc.tile_pool(name="ps", bufs=4, space="PSUM") as pspool,
        ):
            # Loads spread over three DMA queues (2 HWDGE + pool SWDGE);
            # chunk0 split so its first K-tiles land earliest, epack split
            # at 512 so low-range matmuls only wait for the first half:
            #   sync:   c0a(0:256), c0b(256:896), c2,  st0, st2, st4, st5a
            #   scalar: ep_a(0:512), c3, c5,           st1, st3, st5b
            #   pool:   ep_b(512:1024), c1, c4
            comps = [
                cpool.tile([KP, CCOLS], dt_in, name=f"comp{j}")
                for j in range(NCHUNK)
            ]
            epack = cpool.tile([KP, BATCH], dt_in)

            nc.sync.dma_start(out=comps[0][:, 0:256], in_=comp_d[0:KP, 0:256])
            nc.scalar.dma_start(out=epack[:, 0:BANK], in_=epack_d[:, 0:BANK])
            nc.gpsimd.dma_start(
                out=epack[:, BANK:BATCH], in_=epack_d[:, BANK:BATCH]
            )
            nc.sync.dma_start(
                out=comps[0][:, 256:CCOLS], in_=comp_d[0:KP, 256:CCOLS]
            )
            nc.gpsimd.dma_start(
                out=comps[1], in_=comp_d[1 * KP : 2 * KP, :]
            )
            nc.sync.dma_start(out=comps[2], in_=comp_d[2 * KP : 3 * KP, :])
            nc.scalar.dma_start(out=comps[3], in_=comp_d[3 * KP : 4 * KP, :])
            nc.gpsimd.dma_start(
                out=comps[4], in_=comp_d[4 * KP : 5 * KP, :]
            )
            nc.scalar.dma_start(out=comps[5], in_=comp_d[5 * KP : 6 * KP, :])

            for j in range(NCHUNK):
                compj = comps[j]
                last = j == NCHUNK - 1
                out_sb = cpool.tile(
                    [128, BATCH], dt_out, name=f"osb{j}"
                )
                ps = pspool.tile([128, 1024], f32, tag="ps", name=f"ps{j}")
                # one matmul per K-tile and PSUM bank: a single matmul whose
                # output range crosses the 512 bank boundary corrupts its
                # second-bank half on the first (cold p-state) execution, so
                # straddling ranges are split at 512
                for t in range(KTILES):
                    lo, hi = ranges[t], ranges[t + 1]
                    if lo >= hi:
                        continue
                    kh = _kheight(t)
                    for a, b in ((lo, min(hi, BANK)), (max(lo, BANK), hi)):
                        if a >= b:
                            continue
                        nc.tensor.matmul(
                            ps[:, a:b],
                            compj[:kh, t * 128 : (t + 1) * 128],
                            epack[:kh, a:b],
                            start=True,
                            stop=True,
                        )
                if last:
                    # final chunk: per-segment drains on both engines +
                    # per-segment stores for the shortest tail
                    nc.vector.tensor_copy(out=out_sb[:, 0:512], in_=ps[:, 0:512])
                    nc.sync.dma_start(
                        out=out_d[j * 128 : (j + 1) * 128, 0:512],
                        in_=out_sb[:, 0:512],
                    )
                    nc.scalar.copy(out_sb[:, 512:BATCH], ps[:, 512:BATCH])
                    nc.scalar.dma_start(
                        out=out_d[j * 128 : (j + 1) * 128, 512:BATCH],
                        in_=out_sb[:, 512:BATCH],
                    )
                if not last:
                    # one full-chunk [128,1024] drain (f32->fp16 cast here),
                    # alternating the two PSUM-capable engines; the store
                    # rides the matching ring (even -> sync, odd -> scalar)
                    # so its semaphore wait is already resolved when the
                    # queue reaches it and cannot block a later drain
                    if j % 2 == 0:
                        nc.vector.tensor_copy(out=out_sb, in_=ps)
                        nc.sync.dma_start(
                            out=out_d[j * 128 : (j + 1) * 128, :], in_=out_sb
                        )
                    else:
                        nc.scalar.copy(out_sb, ps)
                        nc.scalar.dma_start(
                            out=out_d[j * 128 : (j + 1) * 128, :], in_=out_sb
                        )

    nc.compile()
    return nc


def _get_nc(ranges: tuple):
    if ranges not in _NC_CACHE:
        _NC_CACHE[ranges] = _build_nc(ranges)
    return _NC_CACHE[ranges]


def _prepare(lattent_codes, object_labels, means, components):
    x = np.ascontiguousarray(np.asarray(lattent_codes), dtype=np.float32)
    labels = np.asarray(object_labels).astype(np.int64)
    means = np.ascontiguousarray(np.asarray(means), dtype=np.float32)
    comp = np.ascontiguousarray(np.asarray(components), dtype=np.float32)
    ddt = _np_dtype()

    perm = np.argsort(labels, kind="stable")
    ls = labels[perm]
    xs = x[perm]  # [B, 32]

    counts = np.bincount(ls, minlength=NOBJ)
    cum = np.concatenate([[0], np.cumsum(counts)])
    raw = [int(cum[min(OBJ_PER_KT * t, NOBJ)]) for t in range(KTILES + 1)]
    widths = [raw[t + 1] - raw[t] for t in range(KTILES)]

    # fp32r matmuls need even range starts/widths -> insert zero pad columns
    # (dst_of_src maps sorted column -> padded column; samples pushed to
    # >= BATCH fall off the device batch and are computed on the host).
    # fp16 has no such ISA restriction: no padding at all.
    pad = (lambda w: w % 2) if DTYPE == "f32r" else (lambda w: 0)
    pstart = [0]
    for t in range(KTILES):
        pstart.append(pstart[t] + widths[t] + pad(widths[t]))
    ranges = tuple(min(p, BATCH) for p in pstart[:KTILES]) + (BATCH,)
    dst_of_src = np.concatenate(
        [np.arange(widths[t]) + pstart[t] for t in range(KTILES)]
    )
    on_dev = dst_of_src < BATCH

    # host-side fallback for overflow samples (at most a few, f32r only)
    ov = np.nonzero(~on_dev)[0]
    host_rows = None
    if len(ov):
        host_rows = (
            np.einsum("bp,bpo->bo", xs[ov], comp[ls[ov]]) + means[ls[ov]]
        ).astype(np.float32)

    # Epack[(l%3)*33 + p, dst(i)] = xs[i, p]; row (l%3)*33+32 = 1.0
    band = (ls % OBJ_PER_KT).astype(np.int64)
    epack = np.zeros((KP, BATCH), ddt)
    rows = band[None, on_dev] * ROWS + np.arange(PCA)[:, None]  # [32, n_dev]
    epack[rows, dst_of_src[None, on_dev]] = xs[on_dev].T.astype(ddt)
    epack[band[on_dev] * ROWS + PCA, dst_of_src[on_dev]] = 1.0

    # augmented component table: per object 32 component rows + 1 mean row
    m2 = np.concatenate([comp, means[:, None, :]], axis=1)  # [20, 33, OUT]
    m2 = m2.reshape(NOBJ * ROWS, OUT_DIM)

    in_maps = []
    for c in range(NCORES):
        sl = slice(c * SLICE, (c + 1) * SLICE)
        arr = np.zeros((KP, NCHUNK, KTILES, 128), ddt)
        for t in range(KTILES):
            kh = _kheight(t)
            blk = m2[KP * t : KP * t + kh, sl]  # [kh, 768]
            arr[:kh, :, t, :] = blk.reshape(kh, NCHUNK, 128).astype(ddt)
        # pack per chunk: [NCHUNK*KP, CCOLS], chunk j contiguous
        comp_host = np.ascontiguousarray(
            arr.transpose(1, 0, 2, 3).reshape(NCHUNK * KP, KTILES * 128)
        )
        in_maps.append({"comp": comp_host, "epack": epack})
    return in_maps, ranges, perm, dst_of_src, on_dev, host_rows


def _assemble(results, perm, dst_of_src, on_dev, host_rows):
    out_s = np.empty((BATCH, OUT_DIM), np.float32)
    for c in range(NCORES):
        out_s[on_dev, c * SLICE : (c + 1) * SLICE] = (
            results[c]["out"].astype(np.float32).T[dst_of_src[on_dev]]
        )
    if host_rows is not None:
        out_s[~on_dev] = host_rows
    out = np.empty_like(out_s)
    out[perm] = out_s
    return out


def _output_ok(out, x, labels, means, comp, perm) -> bool:
    """Cheap full-coverage host validation (see module docstring).

    Column-sum invariant catches any corruption wider than fp16 noise in
    any column; 32 exact rows at stride-32 sorted positions catch any
    sorted-contiguous block of >=32 bad samples (the observed failure
    mode).  Both checks cost a few ms on the host.
    """
    s = np.zeros((NOBJ, PCA), np.float32)
    np.add.at(s, labels, x)
    counts = np.bincount(labels, minlength=NOBJ).astype(np.float32)
    exp_colsum = np.einsum("op,opd->d", s, comp) + counts @ means
    got_colsum = out.sum(axis=0, dtype=np.float64)
    if np.abs(got_colsum - exp_colsum).max() > 2.0:
        return False
    idx = perm[np.arange(16, BATCH, 32)]
    e = np.einsum("bp,bpo->bo", x[idx], comp[labels[idx]]) + means[labels[idx]]
    if np.abs(out[idx] - e).max() > 0.15:
        return False
    return True


def run(inputs: dict, trace: bool = False):
    """Run on hardware; returns (full output, BassKernelResults)."""
    from concourse.bass_utils import run_bass_kernel_spmd

    in_maps, ranges, perm, dst_of_src, on_dev, host_rows = _prepare(**inputs)
    nc = _get_nc(ranges)
    res = run_bass_kernel_spmd(
        nc, in_maps, core_ids=list(range(NCORES)), trace=trace
    )
    return _assemble(res.results, perm, dst_of_src, on_dev, host_rows), res


def kernel(lattent_codes, object_labels, means, components) -> np.ndarray:
    inputs = {
        "lattent_codes": lattent_codes,
        "object_labels": object_labels,
        "means": means,
        "components": components,
    }
    x = np.asarray(lattent_codes, dtype=np.float32)
    labels = np.asarray(object_labels).astype(np.int64)
    mns = np.asarray(means, dtype=np.float32)
    comp = np.asarray(components, dtype=np.float32)
    perm = np.argsort(labels, kind="stable")
    out = None
    for _ in range(3):
        out, _res = run(inputs)
        if _output_ok(out, x, labels, mns, comp, perm):
            break
    return out
